# revision 6
# baseline (speedup 1.0000x reference)
"""Trainium2 Bass kernel for pointer-generator final-distribution (scatter_memory).

out[r, v] = p_gens[r] * vocab_ds[r, v]  (+ (1-p_gens[r])*attns[r, l_win]  at
v == sources[l, b(r)], duplicate source ids resolved last-occurrence-wins)

Strategy (8 NeuronCores, SPMD), DMA-roofline bound (~370 GB/s/core across
the 16 shared DMA engines):
  - Shard by batch column: core k owns b in {4k..4k+3}, all T decoder steps
    (rows r = t*B + b). Host pre-gathers rows b-major so device DMAs are
    contiguous; two 128-row groups per core (2 b's x 64 t each).
  - All heavy HBM traffic is bf16: host converts the gathered vocab rows to
    bf16 and the device writes a bf16 output (host upcasts to f32). This
    halves bytes vs f32 streaming; max rel err from the bf16 roundings is
    ~1e-2, under the 2e-2 gate.
  - The scatter is a compact one-hot matmul on the (otherwise idle) PE:
    for each 512-wide subtile, host bakes a [K, 128] block of bf16 update
    values (update k x row, block-diagonal over the two b's) AND the
    matching one-hot [K, 512] rhs (precomputed on host and DMA'd per
    window — cheaper than burning a 512-cycle DVE is_equal per subtile);
    PE computes proj = vals.T @ onehot into f32 PSUM.
  - One DVE scalar_tensor_tensor per subtile fuses the p-gating and the
    scatter-add: out_tile = (vocab_tile * p) + proj. Subtiles with no
    updates take the ACT path (activation Copy with per-partition scale).
  - DMA queues: vocab loads on sync, stores on scalar, one-hot/vals
    prelude loads on gpsimd — separate FIFOs avoid head-of-line blocking.
"""

import numpy as np

N_CORES = 8
WIN = 8192
SUB = 512


def _host_prep(vocab_ds, attns, p_gens, sources, T):
    import ml_dtypes
    bf16 = ml_dtypes.bfloat16
    f32 = np.float32
    vocab_ds = np.ascontiguousarray(vocab_ds, dtype=f32)
    attns = np.ascontiguousarray(attns, dtype=f32)
    p_gens = np.ascontiguousarray(p_gens, dtype=f32)
    src = np.asarray(sources).astype(np.int64)
    rows, V = vocab_ds.shape
    L, B = src.shape
    assert rows == T * B

    ag = (f32(1.0) - p_gens) * attns  # gated copy dist, [rows, L]

    # winners per batch column: duplicate source ids -> last occurrence wins
    wins = []
    for b in range(B):
        d = {}
        col = src[:, b]
        for l in range(L):
            d[int(col[l])] = l
        cols = np.fromiter(d.keys(), dtype=np.int64)
        ls = np.fromiter(d.values(), dtype=np.int64)
        o = np.argsort(cols)
        wins.append((cols[o], ls[o]))

    NW = (V + WIN - 1) // WIN
    # subtile geometry, shared by all cores/groups
    sub_geom = []  # (w, s, c0_abs, width)
    for w in range(NW):
        ww = min(WIN, V - w * WIN)
        for s in range((ww + SUB - 1) // SUB):
            sub_geom.append((w, s, w * WIN + s * SUB, min(SUB, ww - s * SUB)))
    NS = len(sub_geom)
    sub_of = {}
    for i, (w, s, c0, wd) in enumerate(sub_geom):
        sub_of[(w, s)] = i

    BPC = B // N_CORES  # 4
    G = BPC // 2        # 2 groups of 2 b's

    # bucket updates per (core, g, subtile)
    upd = [[[[] for _ in range(NS)] for _ in range(G)] for _ in range(N_CORES)]
    for core in range(N_CORES):
        for g in range(G):
            for half in range(2):
                b = core * BPC + g * 2 + half
                cols, ls = wins[b]
                for c, l in zip(cols.tolist(), ls.tolist()):
                    w = c // WIN
                    s = (c - w * WIN) // SUB
                    i = sub_of[(w, s)]
                    upd[core][g][i].append((half, c, l))

    # uniform-per-(g, subtile) K across cores (one NEFF shared SPMD)
    K_ws = [[max(len(upd[core][g][i]) for core in range(N_CORES)) for i in range(NS)]
            for g in range(G)]
    KMAX = [max(K_ws[g]) if NS else 0 for g in range(G)]
    assert all(k <= 128 for g in range(G) for k in K_ws[g]), \
        "subtile update count exceeds the 128-partition budget"
    # per-(g, window): first subtile index, #subtiles, max K (partition
    # extent of the one-hot load for that window)
    win_info = []
    for g in range(G):
        wi = []
        for w in range(NW):
            idxs = [i for i, (w2, s2, _, _) in enumerate(sub_geom) if w2 == w]
            i0, nsub = idxs[0], len(idxs)
            kw = max(K_ws[g][i] for i in idxs)
            wi.append((i0, nsub, kw))
        win_info.append(wi)

    # per-core device inputs
    in_maps = []
    for core in range(N_CORES):
        m = {}
        for g in range(G):
            row_idx = []
            for half in range(2):
                b = core * BPC + g * 2 + half
                row_idx.extend(t * B + b for t in range(T))
            row_idx = np.asarray(row_idx)
            m[f"vocab{g}"] = vocab_ds[row_idx].astype(bf16)
            m[f"pgen{g}"] = p_gens[row_idx]
            vals = np.zeros((128, NS * 128), dtype=f32)
            oh = np.zeros((128, NS * SUB), dtype=bf16)
            for i in range(NS):
                w, s, c0, wd = sub_geom[i]
                for k, (half, c, l) in enumerate(upd[core][g][i]):
                    # rows of this b occupy partitions half*T .. half*T+T
                    r0 = half * T
                    vals[k, i * 128 + r0: i * 128 + r0 + T] = ag[row_idx[r0: r0 + T], l]
                    oh[k, i * SUB + (c - c0)] = 1.0
            m[f"vals{g}"] = vals.astype(bf16)
            m[f"oh{g}"] = oh
        in_maps.append(m)

    meta = dict(V=V, T=T, B=B, NW=NW, NS=NS, G=G, sub_geom=sub_geom,
                sub_of=sub_of, K_ws=K_ws, KMAX=KMAX, BPC=BPC,
                win_info=win_info)
    return in_maps, meta


def _build_nc(meta):
    from concourse import bacc, mybir

    V, NW, NS, G = meta["V"], meta["NW"], meta["NS"], meta["G"]
    sub_geom, K_ws, KMAX = meta["sub_geom"], meta["K_ws"], meta["KMAX"]
    f32 = mybir.dt.float32

    bf16 = mybir.dt.bfloat16
    nc = bacc.Bacc(None, target_bir_lowering=False, debug=False)
    vocab = [nc.declare_dram_parameter(f"vocab{g}", [128, V], bf16, isOutput=False)
             for g in range(G)]
    pgen = [nc.declare_dram_parameter(f"pgen{g}", [128, 1], f32, isOutput=False)
            for g in range(G)]
    vals = [nc.declare_dram_parameter(f"vals{g}", [128, NS * 128], bf16, isOutput=False)
            for g in range(G)]
    ohp = [nc.declare_dram_parameter(f"oh{g}", [128, NS * SUB], bf16, isOutput=False)
           for g in range(G)]
    out = [nc.declare_dram_parameter(f"out{g}", [128, V], bf16, isOutput=True)
           for g in range(G)]

    from concourse.tile import TileContext

    win_info = meta["win_info"]
    with TileContext(nc) as tc:
        with tc.tile_pool(name="in", bufs=4) as in_pool, \
             tc.tile_pool(name="out", bufs=3) as out_pool, \
             tc.tile_pool(name="small", bufs=1) as small, \
             tc.tile_pool(name="oh", bufs=2) as oh_pool, \
             tc.tile_pool(name="psum", bufs=8, space="PSUM") as psum_pool:

            for g in range(G):
                p_t = small.tile([128, 1], f32, tag=f"p{g}")
                nc.sync.dma_start(out=p_t[:], in_=pgen[g][:])
                kmax = max(wi[2] for wi in win_info[g])
                vals_t = small.tile([128, NS * 128], bf16, tag=f"vals{g}")
                nc.sync.dma_start(out=vals_t[:kmax, :], in_=vals[g][:kmax, :])

                for w in range(NW):
                    c0w = w * WIN
                    ww = min(WIN, V - c0w)
                    i0, nsub, kw = win_info[g][w]
                    t_in = in_pool.tile([128, WIN], bf16, tag="in")
                    nc.sync.dma_start(out=t_in[:, :ww],
                                      in_=vocab[g][:, c0w:c0w + ww])
                    oh_t = oh_pool.tile([128, WIN], bf16, tag="oh")
                    if kw > 0:
                        nc.sync.dma_start(
                            out=oh_t[:kw, :nsub * SUB],
                            in_=ohp[g][:kw, i0 * SUB:(i0 + nsub) * SUB])
                    # p gating on ACT into a fresh tile (no in-place chain:
                    # keeps ACT/DVE/stores decoupled across windows), then
                    # DVE adds the PE scatter projections into the out tile
                    t = out_pool.tile([128, WIN], bf16, tag="out")
                    nc.scalar.activation(
                        t[:, :ww], t_in[:, :ww],
                        mybir.ActivationFunctionType.Copy, scale=p_t[:, :1])
                    for s in range(nsub):
                        i = i0 + s
                        K = K_ws[g][i]
                        if K == 0:
                            continue
                        _, _, c0, wd = sub_geom[i]
                        lo = c0 - c0w
                        ps = psum_pool.tile([128, SUB], f32, tag="ps")
                        nc.tensor.matmul(
                            out=ps[:, :wd],
                            lhsT=vals_t[:K, i * 128:(i + 1) * 128],
                            rhs=oh_t[:K, s * SUB:s * SUB + wd],
                            start=True, stop=True)
                        nc.vector.tensor_add(
                            out=t[:, lo:lo + wd], in0=t[:, lo:lo + wd],
                            in1=ps[:, :wd])
                    nc.gpsimd.dma_start(out=out[g][:, c0w:c0w + ww],
                                        in_=t[:, :ww])
    nc.finalize()
    return nc


def kernel(vocab_ds, attns, p_gens, sources, decoder_batch_len):
    T = int(decoder_batch_len)
    in_maps, meta = _host_prep(vocab_ds, attns, p_gens, sources, T)
    nc = _build_nc(meta)

    from concourse.bass_utils import run_bass_kernel_spmd
    res = run_bass_kernel_spmd(nc, in_maps, list(range(N_CORES)))

    rows, V = np.asarray(vocab_ds).shape
    B, BPC, G = meta["B"], meta["BPC"], meta["G"]
    full = np.empty((rows, V), dtype=np.float32)
    for core in range(N_CORES):
        for g in range(G):
            blk = np.asarray(res.results[core][f"out{g}"], dtype=np.float32)
            for half in range(2):
                b = core * BPC + g * 2 + half
                full[b::B] = blk[half * T:(half + 1) * T]
    return full


# revision 8
# speedup vs baseline: 1.0139x; 1.0139x over previous
"""Trainium2 Bass kernel for pointer-generator final-distribution (scatter_memory).

out[r, v] = p_gens[r] * vocab_ds[r, v]  (+ (1-p_gens[r])*attns[r, l_win]  at
v == sources[l, b(r)], duplicate source ids resolved last-occurrence-wins)

Strategy (8 NeuronCores, SPMD), DMA-roofline bound (~360 GB/s/core across
the 16 shared DMA engines):
  - Shard by batch column: core k owns b in {4k..4k+3}, all T decoder steps
    (rows r = t*B + b). Host pre-gathers rows b-major so device DMAs are
    contiguous; two 128-row groups per core (2 b's x 64 t each). The two
    group streams are interleaved window-by-window so neither drains while
    the other warms up.
  - All heavy HBM traffic is bf16 (host converts in, upcasts out) — halves
    bytes vs f32; max rel err ~1e-2 vs the 2e-2 gate. DRAM buffers are laid
    out window-major (each [128, 8192] window block contiguous) so every
    DMA is a linear ~2MB HBM sweep of full-width 16KB descriptors.
  - The scatter is a compact one-hot matmul on the otherwise-idle PE: per
    512-wide subtile the host bakes [K, 128] bf16 update values
    (block-diagonal over the two b's) and the matching one-hot [K, 512]
    rhs (precomputed host-side — cheaper than a 512-cycle DVE is_equal per
    subtile); PE computes proj = vals.T @ onehot into f32 PSUM.
  - ACT applies the p gating into a fresh out tile (keeps the act/add/store
    chain out-of-place so engines decouple across windows); the PSUM adds
    alternate DVE <-> Pool per subtile. Loads dispatch from sync, stores
    from Pool — no engine has a store wait blocking later load dispatches.
"""

import numpy as np

N_CORES = 8
WIN = 8192
SUB = 512
KH = 32  # one-hot DRAM rows reserved per window block


def _host_prep(vocab_ds, attns, p_gens, sources, T):
    import ml_dtypes
    bf16 = ml_dtypes.bfloat16
    f32 = np.float32
    vocab_ds = np.ascontiguousarray(vocab_ds, dtype=f32)
    attns = np.ascontiguousarray(attns, dtype=f32)
    p_gens = np.ascontiguousarray(p_gens, dtype=f32)
    src = np.asarray(sources).astype(np.int64)
    rows, V = vocab_ds.shape
    L, B = src.shape
    assert rows == T * B

    ag = (f32(1.0) - p_gens) * attns  # gated copy dist, [rows, L]

    # winners per batch column: duplicate source ids -> last occurrence wins
    wins = []
    for b in range(B):
        d = {}
        col = src[:, b]
        for l in range(L):
            d[int(col[l])] = l
        cols = np.fromiter(d.keys(), dtype=np.int64)
        ls = np.fromiter(d.values(), dtype=np.int64)
        o = np.argsort(cols)
        wins.append((cols[o], ls[o]))

    NW = (V + WIN - 1) // WIN
    # subtile geometry, shared by all cores/groups
    sub_geom = []  # (w, s, c0_abs, width)
    for w in range(NW):
        ww = min(WIN, V - w * WIN)
        for s in range((ww + SUB - 1) // SUB):
            sub_geom.append((w, s, w * WIN + s * SUB, min(SUB, ww - s * SUB)))
    NS = len(sub_geom)
    sub_of = {}
    for i, (w, s, c0, wd) in enumerate(sub_geom):
        sub_of[(w, s)] = i

    BPC = B // N_CORES  # 4
    G = BPC // 2        # 2 groups of 2 b's

    # bucket updates per (core, g, subtile)
    upd = [[[[] for _ in range(NS)] for _ in range(G)] for _ in range(N_CORES)]
    for core in range(N_CORES):
        for g in range(G):
            for half in range(2):
                b = core * BPC + g * 2 + half
                cols, ls = wins[b]
                for c, l in zip(cols.tolist(), ls.tolist()):
                    w = c // WIN
                    s = (c - w * WIN) // SUB
                    i = sub_of[(w, s)]
                    upd[core][g][i].append((half, c, l))

    # uniform-per-(g, subtile) K across cores (one NEFF shared SPMD)
    K_ws = [[max(len(upd[core][g][i]) for core in range(N_CORES)) for i in range(NS)]
            for g in range(G)]
    assert all(k <= 128 for g in range(G) for k in K_ws[g]), \
        "subtile update count exceeds the 128-partition budget"
    # per-(g, window): first subtile index, #subtiles, max K (partition
    # extent of the one-hot load for that window)
    win_info = []
    for g in range(G):
        wi = []
        for w in range(NW):
            idxs = [i for i, (w2, s2, _, _) in enumerate(sub_geom) if w2 == w]
            i0, nsub = idxs[0], len(idxs)
            kw = max(K_ws[g][i] for i in idxs)
            assert kw <= KH
            wi.append((i0, nsub, kw))
        win_info.append(wi)
    max_nsub = max(wi[1] for g in range(G) for wi in win_info[g])

    # per-core device inputs (window-major blocked layouts)
    in_maps = []
    for core in range(N_CORES):
        m = {}
        for g in range(G):
            row_idx = []
            for half in range(2):
                b = core * BPC + g * 2 + half
                row_idx.extend(t * B + b for t in range(T))
            row_idx = np.asarray(row_idx)
            vg = vocab_ds[row_idx].astype(bf16)
            vb = np.zeros((NW * 128, WIN), dtype=bf16)
            for w in range(NW):
                c0w = w * WIN
                ww = min(WIN, V - c0w)
                vb[w * 128:w * 128 + 128, :ww] = vg[:, c0w:c0w + ww]
            m[f"vocab{g}"] = vb
            m[f"pgen{g}"] = p_gens[row_idx]
            vals = np.zeros((128, NS * 128), dtype=f32)
            oh = np.zeros((NW * KH, max_nsub * SUB), dtype=bf16)
            for i in range(NS):
                w, s, c0, wd = sub_geom[i]
                for k, (half, c, l) in enumerate(upd[core][g][i]):
                    # rows of this b occupy partitions half*T .. half*T+T
                    r0 = half * T
                    vals[k, i * 128 + r0: i * 128 + r0 + T] = ag[row_idx[r0: r0 + T], l]
                    oh[w * KH + k, s * SUB + (c - c0)] = 1.0
            m[f"vals{g}"] = vals.astype(bf16)
            m[f"oh{g}"] = oh
        in_maps.append(m)

    meta = dict(V=V, T=T, B=B, NW=NW, NS=NS, G=G, sub_geom=sub_geom,
                sub_of=sub_of, K_ws=K_ws, BPC=BPC, win_info=win_info,
                max_nsub=max_nsub)
    return in_maps, meta


def _build_nc(meta):
    from concourse import bacc, mybir

    V, NW, NS, G = meta["V"], meta["NW"], meta["NS"], meta["G"]
    sub_geom, K_ws = meta["sub_geom"], meta["K_ws"]
    max_nsub = meta["max_nsub"]
    f32 = mybir.dt.float32

    bf16 = mybir.dt.bfloat16
    nc = bacc.Bacc(None, target_bir_lowering=False, debug=False)
    vocab = [nc.declare_dram_parameter(f"vocab{g}", [NW * 128, WIN], bf16,
                                       isOutput=False) for g in range(G)]
    pgen = [nc.declare_dram_parameter(f"pgen{g}", [128, 1], f32, isOutput=False)
            for g in range(G)]
    vals = [nc.declare_dram_parameter(f"vals{g}", [128, NS * 128], bf16, isOutput=False)
            for g in range(G)]
    ohp = [nc.declare_dram_parameter(f"oh{g}", [NW * KH, max_nsub * SUB], bf16,
                                     isOutput=False) for g in range(G)]
    out = [nc.declare_dram_parameter(f"out{g}", [NW * 128, WIN], bf16, isOutput=True)
           for g in range(G)]

    from concourse.tile import TileContext

    win_info = meta["win_info"]
    with TileContext(nc) as tc:
        with tc.tile_pool(name="in", bufs=3) as in_pool, \
             tc.tile_pool(name="out", bufs=4) as out_pool, \
             tc.tile_pool(name="small", bufs=1) as small, \
             tc.tile_pool(name="oh", bufs=2) as oh_pool, \
             tc.tile_pool(name="psum", bufs=8, space="PSUM") as psum_pool:

            p_t, vals_t = [], []
            for g in range(G):
                pt = small.tile([128, 1], f32, tag=f"p{g}")
                nc.sync.dma_start(out=pt[:], in_=pgen[g][:])
                p_t.append(pt)
                kmax = max(wi[2] for wi in win_info[g])
                vt = small.tile([128, NS * 128], bf16, tag=f"vals{g}")
                nc.sync.dma_start(out=vt[:kmax, :], in_=vals[g][:kmax, :])
                vals_t.append(vt)

            for w in range(NW):
                c0w = w * WIN
                ww = min(WIN, V - c0w)
                for g in range(G):
                    i0, nsub, kw = win_info[g][w]
                    t_in = in_pool.tile([128, WIN], bf16, tag="in")
                    nc.sync.dma_start(out=t_in[:, :ww],
                                      in_=vocab[g][w * 128:w * 128 + 128, :ww])
                    oh_t = oh_pool.tile([128, WIN], bf16, tag="oh")
                    if kw > 0:
                        nc.sync.dma_start(
                            out=oh_t[:kw, :nsub * SUB],
                            in_=ohp[g][w * KH:w * KH + kw, :nsub * SUB])
                    t = out_pool.tile([128, WIN], bf16, tag="out")
                    nc.scalar.activation(
                        t[:, :ww], t_in[:, :ww],
                        mybir.ActivationFunctionType.Copy, scale=p_t[g][:, :1])
                    for s in range(nsub):
                        i = i0 + s
                        K = K_ws[g][i]
                        if K == 0:
                            continue
                        _, _, c0, wd = sub_geom[i]
                        lo = c0 - c0w
                        ps = psum_pool.tile([128, SUB], f32, tag="ps")
                        nc.tensor.matmul(
                            out=ps[:, :wd],
                            lhsT=vals_t[g][:K, i * 128:(i + 1) * 128],
                            rhs=oh_t[:K, s * SUB:s * SUB + wd],
                            start=True, stop=True)
                        nc.vector.tensor_add(
                            out=t[:, lo:lo + wd], in0=t[:, lo:lo + wd],
                            in1=ps[:, :wd])
                    nc.gpsimd.dma_start(out=out[g][w * 128:w * 128 + 128, :ww],
                                        in_=t[:, :ww])
    nc.finalize()
    return nc


def _unshard(res, meta, rows, V):
    B, BPC, G, T, NW = meta["B"], meta["BPC"], meta["G"], meta["T"], meta["NW"]
    full = np.empty((rows, V), dtype=np.float32)
    for core in range(N_CORES):
        for g in range(G):
            blk = np.asarray(res.results[core][f"out{g}"], dtype=np.float32)
            flat = np.empty((128, V), dtype=np.float32)
            for w in range(NW):
                c0w = w * WIN
                ww = min(WIN, V - c0w)
                flat[:, c0w:c0w + ww] = blk[w * 128:w * 128 + 128, :ww]
            for half in range(2):
                b = core * BPC + g * 2 + half
                full[b::B] = flat[half * T:(half + 1) * T]
    return full


def kernel(vocab_ds, attns, p_gens, sources, decoder_batch_len):
    T = int(decoder_batch_len)
    in_maps, meta = _host_prep(vocab_ds, attns, p_gens, sources, T)
    nc = _build_nc(meta)

    from concourse.bass_utils import run_bass_kernel_spmd
    res = run_bass_kernel_spmd(nc, in_maps, list(range(N_CORES)))

    rows, V = np.asarray(vocab_ds).shape
    return _unshard(res, meta, rows, V)
